# revision 50
# baseline (speedup 1.0000x reference)
"""AttnBlock (GroupNorm + single-head full attention + residual) on 8 trn2 cores.

Sharding: core c in 0..7 handles batch b = c//4, query-block qb = c%4 (1024 of
4096 positions). Each core receives its batch's x with columns rotated so its
query block sits at columns 0:1023 (attention and groupnorm statistics are
invariant to a consistent permutation of key positions), computes the full
groupnorm + K/V for all 4096 positions, attention for its 1024 query positions,
and returns out[512, 1024]. The host gathers the 8 blocks.

On-device pipeline (all matmuls bf16 with fp32 PSUM accumulation):
  1. Stream x (fp32) through SBUF: per-channel sum / sum-of-squares for
     groupnorm stats (fp32), cast x to bf16 for the matmul path.
  2. Group stats via tiny one-hot matmuls across partitions; groupnorm is then
     folded into the QKV weights: h = a*x + bb  =>  W' = W * a (per input
     channel), bias' = W @ bb (+ original conv bias).
  3. q = W_q' x  [c, 1024];  k = W_k' x  [c, 4096];  vT = x^T W_v' [j, c]
     (v produced pre-transposed so the attention contraction over j needs no
     transposes anywhere).
  4. Per 512-wide query chunk: scoresT[j, i] = k^T q accumulated per 128-row
     j-tile in PSUM, exp on the scalar engine (softmax max-subtraction is
     skipped: logits are O(5) by construction), sum_j exp via ones-matmul,
     attn0[c, i] = vT^T p accumulated over all 32 j-tiles in PSUM.
  5. attn = attn0 / sum + v-path bias; proj = W_p attn + p_b + x (residual
     re-read from DRAM in fp32).
"""

import os
import sys

import numpy as np

for _p in ("/opt/trn_rl_repo", "/root/.axon_site/_ro/trn_rl_repo"):
    if os.path.isdir(_p) and _p not in sys.path:
        sys.path.insert(0, _p)

import ml_dtypes  # noqa: E402

import concourse.bacc as bacc  # noqa: E402
import concourse.bass as bass  # noqa: E402
import concourse.mybir as mybir  # noqa: E402
import concourse.tile as tile  # noqa: E402

F32 = mybir.dt.float32
BF16 = mybir.dt.bfloat16
FP8 = mybir.dt.float8e4
# fp8 attention-value path: p and vT quantized to e4m3, attnV + sumexp
# matmuls run in DoubleRow mode (2 contraction rows per PE cell -> half the
# matmul time). exp is biased by EXP_SHIFT so p fits e4m3 range; the shift
# cancels exactly in the softmax normalization.
FP8_ATTN = True
EXP_SHIFT = -2.0
AF = mybir.ActivationFunctionType
AX = mybir.AxisListType

P = 128
C = 512
CT = C // P            # 4 channel tiles
N = 4096               # key/value positions per batch
NQ = 1024              # query positions per core
ICH = 512              # query chunk (PSUM free dim)
NIC = NQ // ICH        # 2 query chunks
JT = N // P            # 32 key j-tiles
JC = N // 512          # 8 key j-chunks
NG = 32                # groupnorm groups
GS = C // NG           # 16 channels per group
EPS = 1e-6
NE = GS * N            # elements per group
SCALE = float(C) ** -0.5


def _emit(nc, tc, io):
    ctx = tc  # alias
    from contextlib import ExitStack

    es = ExitStack()
    wpool = es.enter_context(tc.tile_pool(name="w", bufs=4))
    cpool = es.enter_context(tc.tile_pool(name="consts", bufs=1))
    spool = es.enter_context(tc.tile_pool(name="stat", bufs=1))
    xbpool = es.enter_context(tc.tile_pool(name="xb", bufs=CT))
    kpool = es.enter_context(tc.tile_pool(name="k", bufs=CT))
    vpool = es.enter_context(tc.tile_pool(name="vt", bufs=JT))
    qpool = es.enter_context(tc.tile_pool(name="q", bufs=CT))
    sqpool = es.enter_context(tc.tile_pool(name="sq", bufs=2))
    ppool = es.enter_context(tc.tile_pool(name="p", bufs=4))
    apool = es.enter_context(tc.tile_pool(name="attn", bufs=8))
    anpool = es.enter_context(tc.tile_pool(name="anorm", bufs=2))
    rpool = es.enter_context(tc.tile_pool(name="rn", bufs=2))
    opool = es.enter_context(tc.tile_pool(name="osb", bufs=4))
    respool = es.enter_context(tc.tile_pool(name="res", bufs=1))
    psmm = es.enter_context(tc.tile_pool(name="psmm", bufs=4, space="PSUM"))
    pssc = es.enter_context(tc.tile_pool(name="pssc", bufs=3, space="PSUM"))
    pssum = es.enter_context(tc.tile_pool(name="pssum", bufs=1, space="PSUM"))

    xb16 = io["xb16"]
    xres = io["xres"]
    out = io["out"]

    # ---- phase B: x tiles first on the SP HWDGE queue (startup-critical);
    # everything else via gpsimd's software DGE so neither the SP queue nor
    # the ACT sequencer blocks on DMA ring credits.
    xb_sb = []
    s_tiles = []
    H = N // 2
    # x split between the SP HWDGE queue and gpsimd's SWDGE rings — both are
    # compute-free sequencers. The ACT queue must issue NO input DMAs: its
    # ring-credit waits would block all scalar-engine compute behind them.
    # 8 half-tiles over three rings (SP, ACT, SWDGE). The ACT queue gets only
    # 3 early DMAs — more would hit ring-credit waits that stall ACT compute.
    ring = [nc.sync, nc.scalar, nc.gpsimd,
            nc.sync, nc.scalar, nc.gpsimd,
            nc.sync, nc.scalar]
    for t in range(CT):
        xb = xbpool.tile([P, N], BF16, tag="xb", name=f"xb{t}")
        ring[2 * t].dma_start(xb[:, :H], xb16[t * P:(t + 1) * P, :H])
        ring[2 * t + 1].dma_start(xb[:, H:], xb16[t * P:(t + 1) * P, H:])
        xb_sb.append(xb)

    # ---- constants: small ones first (the stats matmuls need G early),
    # then the 4MB of weights, then the residual ---------------------------
    G_dma = cpool.tile([P, CT * NG], F32, tag="Gmd", name="Gmd")
    nc.sync.dma_start(G_dma, io["gmask"][:, :])
    G_sb = cpool.tile([P, CT * NG], F32, tag="Gm", name="Gm")
    # NOTE: the ACT copy of G is emitted AFTER the stats loop — engine streams
    # run in emission order, and an early-emitted copy waiting on the G DMA
    # (queued behind 4MB of x) would stall every ACT square behind it.
    GT_dma = cpool.tile([NG, C], F32, tag="GTmd", name="GTmd")
    nc.gpsimd.dma_start(GT_dma, io["gtmask"][:, :])
    GT_sb = cpool.tile([NG, C], F32, tag="GTm", name="GTm")
    nc.vector.tensor_copy(GT_sb, GT_dma)
    bias_all = cpool.tile([P, 24], F32, tag="bias_all", name="bias_all")
    nc.sync.dma_start(bias_all, io["bias6"][:, :])
    w_sb = {}
    for i, wn in enumerate(("wq", "wk", "wv", "wp")):
        wt = wpool.tile([P, CT, C], BF16, tag="w", name=f"{wn}_all")
        eng = nc.sync if i % 2 == 0 else nc.gpsimd
        eng.dma_start(wt, io[wn].rearrange("(t p) o -> p t o", p=P))
        w_sb[wn] = [wt[:, t, :] for t in range(CT)]
    # residual: DRAM-only dependency, needed only at the proj epilogue
    res_all = respool.tile([P, CT, NIC, ICH], F32, tag="res", name="res_all")
    nc.gpsimd.dma_start(
        res_all, xres.rearrange("(t p) (i n) -> p t i n", p=P, n=ICH))
    res_sb = [res_all[:, t, ic, :] for ic in range(NIC) for t in range(CT)]
    small = {}
    for idx, nm in enumerate(("qb2", "kb2", "vb2", "pb2", "gnw2", "gnb2")):
        small[nm] = bias_all[:, idx * CT:(idx + 1) * CT]
    ones_b = cpool.tile([P, 1], BF16, tag="ones_b", name="ones_b")
    nc.vector.memset(ones_b, 1.0)
    ones_p_t = cpool.tile([P, 2, 16], FP8, tag="ones_p", name="ones_p")
    nc.vector.memset(ones_p_t, 1.0)
    ones_p = ones_p_t[:, :, 0:1]  # pair stride 16 (DoubleRow needs step%16==0)
    nshift = cpool.tile([P, 1], F32, tag="nshift", name="nshift")
    nc.vector.memset(nshift, EXP_SHIFT)

    # ---- stats per half-tile (chases the DMA halves as they land) -------
    # s1 via DVE tensor_scalar+accum (bf16 2x mode, ~2x faster than reduce);
    # squares on ACT except the last tile's, which go to DVE STT so the two
    # engines finish together.
    # hs layout per tile: [s1_h0, s2_h0, s1_h1, s2_h1] so each half's (s1,s2)
    # pair feeds the group-stats matmul directly (no combining adds on the
    # critical chain). The last tile's s1 runs on ACT to balance the engines.
    for t in range(CT):
        xb = xb_sb[t]
        hs = spool.tile([P, 4], F32, tag=f"hs{t}", name=f"hs{t}")
        for h in range(2):
            hsl = slice(h * H, (h + 1) * H)
            sq_scr = sqpool.tile([P, H], BF16, tag="sq", name=f"sq{t}_{h}")
            nc.scalar.activation(sq_scr, xb[:, hsl], AF.Square,
                                 accum_out=hs[:, 2 * h + 1:2 * h + 2])
            s1_scr = sqpool.tile([P, H], BF16, tag="s1s", name=f"s1s{t}_{h}")
            if t == CT - 1:
                nc.scalar.activation(s1_scr, xb[:, hsl], AF.Copy,
                                     accum_out=hs[:, 2 * h:2 * h + 1])
            else:
                nc.vector.tensor_scalar(
                    s1_scr, xb[:, hsl], 1.0, 0.0, mybir.AluOpType.mult,
                    mybir.AluOpType.add, accum_out=hs[:, 2 * h:2 * h + 1])
        s_tiles.append(hs)
    nc.scalar.copy(G_sb, G_dma)

    # ---- phase C: group stats (accumulate all 8 half-contributions) -----
    gs_ps = psmm.tile([NG, 2], F32, tag="mm", name="gsums")
    for t in range(CT):
        for h in range(2):
            nc.tensor.matmul(gs_ps, lhsT=G_sb[:, t * NG:(t + 1) * NG],
                             rhs=s_tiles[t][:, 2 * h:2 * h + 2],
                             start=(t == 0 and h == 0),
                             stop=(t == CT - 1 and h == 1))
    vals = spool.tile([NG, 2], F32, tag="vals", name="vals")  # col0 rsig col1 mu
    ex2 = spool.tile([NG, 1], F32, tag="ex2", name="ex2")
    msq = spool.tile([NG, 1], F32, tag="msq", name="msq")
    sd = spool.tile([NG, 1], F32, tag="sd", name="sd")
    nc.vector.tensor_scalar_mul(vals[:, 1:2], gs_ps[:, 0:1], 1.0 / NE)
    nc.vector.tensor_scalar_mul(ex2, gs_ps[:, 1:2], 1.0 / NE)
    nc.vector.tensor_mul(msq, vals[:, 1:2], vals[:, 1:2])
    nc.vector.tensor_sub(msq, ex2, msq)
    nc.vector.tensor_scalar_add(msq, msq, EPS)
    nc.scalar.activation(sd, msq, AF.Sqrt)
    nc.vector.reciprocal_approx_fast(vals[:, 0:1], sd)

    # ---- phase D: per-channel a/bb, fold into weights -------------------
    a_t, bbb_t = [], []
    for t in range(CT):
        ch = psmm.tile([P, 2], F32, tag="mm", name=f"ch{t}")
        nc.tensor.matmul(ch, lhsT=GT_sb[:, t * P:(t + 1) * P], rhs=vals,
                         start=True, stop=True)
        at = spool.tile([P, 1], F32, tag=f"a{t}", name=f"a{t}")
        nc.vector.tensor_mul(at, ch[:, 0:1], small["gnw2"][:, t:t + 1])
        mt = spool.tile([P, 1], F32, tag=f"mt{t}", name=f"mt{t}")
        nc.vector.tensor_mul(mt, ch[:, 1:2], at)
        bbf = spool.tile([P, 1], F32, tag=f"bbf{t}", name=f"bbf{t}")
        nc.vector.tensor_sub(bbf, small["gnb2"][:, t:t + 1], mt)
        bbb = spool.tile([P, 1], BF16, tag=f"bbb{t}", name=f"bbb{t}")
        nc.vector.tensor_copy(bbb, bbf)
        a_t.append(at)
        bbb_t.append(bbb)

    # bias' = W @ bb (+ host conv bias); must read W before in-place scaling
    biases = {}
    for wn, hb in (("wq", "qb2"), ("wk", "kb2"), ("wv", "vb2")):
        bl = []
        for t in range(CT):
            bp = psmm.tile([P, 1], F32, tag="mm", name=f"B{wn}{t}")
            for ct in range(CT):
                nc.tensor.matmul(bp, lhsT=w_sb[wn][ct][:, t * P:(t + 1) * P],
                                 rhs=bbb_t[ct], start=(ct == 0),
                                 stop=(ct == CT - 1))
            bt = spool.tile([P, 1], F32, tag=f"bi{wn}{t}", name=f"bi{wn}{t}")
            nc.vector.tensor_add(bt, bp, small[hb][:, t:t + 1])
            bl.append(bt)
        biases[wn] = bl
    for wn in ("wq", "wk", "wv"):
        for ct in range(CT):
            nc.vector.tensor_scalar_mul(w_sb[wn][ct], w_sb[wn][ct], a_t[ct])

    # ---- phase E: q, then (k, vT) j-chunk-major -------------------------
    q_sb = [qpool.tile([P, NQ], BF16, tag="q", name=f"q{t}") for t in range(CT)]
    for t in range(CT):
        for ic in range(NIC):
            qp = psmm.tile([P, ICH], F32, tag="mm", name=f"qp{t}_{ic}")
            for ct in range(CT):
                nc.tensor.matmul(qp, lhsT=w_sb["wq"][ct][:, t * P:(t + 1) * P],
                                 rhs=xb_sb[ct][:, ic * ICH:(ic + 1) * ICH],
                                 start=(ct == 0), stop=(ct == CT - 1))
            nc.scalar.activation(q_sb[t][:, ic * ICH:(ic + 1) * ICH], qp,
                                 AF.Identity, bias=biases["wq"][t])
    k_sb = [kpool.tile([P, N], BF16, tag="k", name=f"k{t}") for t in range(CT)]
    vT_sb = []
    for jc in range(JC):
        sl = slice(jc * 512, (jc + 1) * 512)
        for t in range(CT):
            kp = psmm.tile([P, 512], F32, tag="mm", name=f"kp{t}_{jc}")
            for ct in range(CT):
                nc.tensor.matmul(kp, lhsT=w_sb["wk"][ct][:, t * P:(t + 1) * P],
                                 rhs=xb_sb[ct][:, sl],
                                 start=(ct == 0), stop=(ct == CT - 1))
            nc.scalar.activation(k_sb[t][:, sl], kp, AF.Identity,
                                 bias=biases["wk"][t])
        for jj in range(4):
            j = jc * 4 + jj
            vp = psmm.tile([P, C], F32, tag="mm", name=f"vp{j}")
            for ct in range(CT):
                nc.tensor.matmul(vp, lhsT=xb_sb[ct][:, j * P:(j + 1) * P],
                                 rhs=w_sb["wv"][ct],
                                 start=(ct == 0), stop=(ct == CT - 1))
            if FP8_ATTN:
                if j % 2 == 0:
                    vt = vpool.tile([P, 2, C], FP8, tag="vt", name=f"vt{j // 2}")
                    vT_sb.append(vt)
                nc.vector.tensor_copy(vT_sb[j // 2][:, j % 2, :], vp)
            else:
                vt = vpool.tile([P, C], BF16, tag="vt", name=f"vt{j}")
                nc.vector.tensor_copy(vt, vp)
                vT_sb.append(vt)

    # ---- phase F: attention per query chunk -----------------------------
    DR = mybir.MatmulPerfMode.DoubleRow
    attn_sb = [[None] * CT for _ in range(NIC)]
    for ic in range(NIC):
        isl = slice(ic * ICH, (ic + 1) * ICH)
        att_ps = [psmm.tile([P, ICH], F32, tag="mm", name=f"att{ic}_{c}")
                  for c in range(CT)]
        se_ps = pssum.tile([1, ICH], F32, tag="se", name=f"se{ic}")
        if FP8_ATTN:
            # Software-pipelined: emit pair g+1's scores before pair g's
            # DoubleRow matmuls. The DR ldweights carry the wait on exp(g)
            # (Bacc moves matmul waits to ldweights), and the PE is in-order,
            # so without the pipeline it idles ~exp-latency every pair.
            NPAIR = JT // 2
            pg_tiles = {}

            def emit_scores(g):
                pg = ppool.tile([P, 2, ICH], FP8, tag="p", name=f"p{ic}_{g}")
                for r in range(2):
                    j = 2 * g + r
                    sp = pssc.tile([P, ICH], F32, tag="sc", name=f"sp{ic}_{j}")
                    for ct in range(CT):
                        nc.tensor.matmul(
                            sp, lhsT=k_sb[ct][:, j * P:(j + 1) * P],
                            rhs=q_sb[ct][:, isl],
                            start=(ct == 0), stop=(ct == CT - 1))
                    nc.scalar.activation(pg[:, r, :], sp, AF.Exp,
                                         bias=nshift, scale=SCALE)
                pg_tiles[g] = pg

            emit_scores(0)
            for g in range(NPAIR):
                if g + 1 < NPAIR:
                    emit_scores(g + 1)
                pg = pg_tiles.pop(g)
                nc.tensor.matmul(se_ps, lhsT=ones_p, rhs=pg, perf_mode=DR,
                                 start=(g == 0), stop=(g == NPAIR - 1))
                for c in range(CT):
                    nc.tensor.matmul(
                        att_ps[c], lhsT=vT_sb[g][:, :, c * P:(c + 1) * P],
                        rhs=pg, perf_mode=DR,
                        start=(g == 0), stop=(g == NPAIR - 1))
        else:
            for j in range(JT):
                sp = pssc.tile([P, ICH], F32, tag="sc", name=f"sp{ic}_{j}")
                for ct in range(CT):
                    nc.tensor.matmul(sp, lhsT=k_sb[ct][:, j * P:(j + 1) * P],
                                     rhs=q_sb[ct][:, isl],
                                     start=(ct == 0), stop=(ct == CT - 1))
                pj = ppool.tile([P, ICH], BF16, tag="p", name=f"p{ic}_{j}")
                nc.scalar.activation(pj, sp, AF.Exp, scale=SCALE)
                nc.tensor.matmul(se_ps, lhsT=ones_b, rhs=pj,
                                 start=(j == 0), stop=(j == JT - 1))
                for c in range(CT):
                    nc.tensor.matmul(att_ps[c],
                                     lhsT=vT_sb[j][:, c * P:(c + 1) * P],
                                     rhs=pj, start=(j == 0), stop=(j == JT - 1))
        r_sb = rpool.tile([1, ICH], F32, tag="r", name=f"r{ic}")
        nc.vector.reciprocal_approx_fast(r_sb, se_ps)
        # [1,512] -> [128,512] partition broadcast on gpsimd (keeps PE free)
        rbc = rpool.tile([P, ICH], F32, tag="rbc", name=f"rbc{ic}")
        nc.gpsimd.partition_broadcast(rbc, r_sb)
        for c in range(CT):
            an = anpool.tile([P, ICH], F32, tag="an", name=f"an{ic}_{c}")
            nc.vector.tensor_mul(an, att_ps[c], rbc)
            at = apool.tile([P, ICH], BF16, tag="attn", name=f"at{ic}_{c}")
            nc.scalar.activation(at, an, AF.Identity, bias=biases["wv"][c])
            attn_sb[ic][c] = at

    # ---- phase G: proj + residual + store -------------------------------
    for ic in range(NIC):
        isl = slice(ic * ICH, (ic + 1) * ICH)
        for t in range(CT):
            op_ps = pssc.tile([P, ICH], F32, tag="sc", name=f"op{ic}_{t}")
            for ct in range(CT):
                nc.tensor.matmul(op_ps, lhsT=w_sb["wp"][ct][:, t * P:(t + 1) * P],
                                 rhs=attn_sb[ic][ct],
                                 start=(ct == 0), stop=(ct == CT - 1))
            osb = opool.tile([P, ICH], F32, tag="o", name=f"o{ic}_{t}")
            nc.vector.scalar_tensor_tensor(
                osb, in0=op_ps, scalar=small["pb2"][:, t:t + 1],
                in1=res_sb[ic * CT + t],
                op0=mybir.AluOpType.add, op1=mybir.AluOpType.add)
            eng = nc.sync if t % 2 == 0 else nc.scalar
            eng.dma_start(out[t * P:(t + 1) * P, isl], osb)
    es.close()


def build_nc():
    nc = bacc.Bacc("TRN2", target_bir_lowering=False, debug=False)
    io = {}
    io["xb16"] = nc.dram_tensor("xb16", [C, N], BF16, kind="ExternalInput").ap()
    io["xres"] = nc.dram_tensor("xres", [C, NQ], F32, kind="ExternalInput").ap()
    for wn in ("wq", "wk", "wv", "wp"):
        io[wn] = nc.dram_tensor(wn, [C, C], BF16, kind="ExternalInput").ap()
    io["bias6"] = nc.dram_tensor("bias6", [P, 24], F32,
                                 kind="ExternalInput").ap()
    io["gmask"] = nc.dram_tensor("gmask", [P, CT * NG], F32,
                                 kind="ExternalInput").ap()
    io["gtmask"] = nc.dram_tensor("gtmask", [NG, C], F32,
                                  kind="ExternalInput").ap()
    io["out"] = nc.dram_tensor("out", [C, NQ], F32, kind="ExternalOutput").ap()
    with tile.TileContext(nc) as tc:
        _emit(nc, tc, io)
    nc.compile()
    return nc


def make_in_maps(inputs):
    bf = ml_dtypes.bfloat16
    x = np.asarray(inputs["x"], np.float32)
    B = x.shape[0]
    bias6 = np.concatenate(
        [np.asarray(inputs[nm], np.float32).reshape(CT, P).T
         for nm in ("q_b", "k_b", "v_b", "p_b", "gn_w", "gn_b")], axis=1)
    shared = {
        "wq": np.ascontiguousarray(np.asarray(inputs["q_w"], np.float32).T).astype(bf),
        "wk": np.ascontiguousarray(np.asarray(inputs["k_w"], np.float32).T).astype(bf),
        "wv": np.ascontiguousarray(np.asarray(inputs["v_w"], np.float32).T).astype(bf),
        "wp": np.ascontiguousarray(np.asarray(inputs["p_w"], np.float32).T).astype(bf),
        "bias6": np.ascontiguousarray(bias6),
    }
    # one-hot group masks: channel k of c-tile t belongs to group (t*128+k)//16
    gm = np.zeros((P, CT, NG), np.float32)
    for t in range(CT):
        for k in range(P):
            gm[k, t, (t * P + k) // GS] = 1.0
    shared["gmask"] = np.ascontiguousarray(gm.reshape(P, CT * NG))
    gt = np.zeros((NG, C), np.float32)
    for ch in range(C):
        gt[ch // GS, ch] = 1.0
    shared["gtmask"] = gt
    in_maps = []
    for core in range(8):
        b, qb = core // 4, core % 4
        xb = x[b].reshape(C, N)
        xp = np.ascontiguousarray(np.roll(xb, -qb * NQ, axis=1))
        in_maps.append({**shared,
                        "xb16": xp.astype(bf),
                        "xres": np.ascontiguousarray(xp[:, :NQ])})
    return in_maps


_NC_CACHE = {}


def run_cores(inputs, trace=False, **kw):
    from concourse.bass_utils import run_bass_kernel_spmd
    if "nc" not in _NC_CACHE:
        _NC_CACHE["nc"] = build_nc()
    nc = _NC_CACHE["nc"]
    in_maps = make_in_maps(inputs)
    res = run_bass_kernel_spmd(nc, in_maps, core_ids=list(range(8)),
                               trace=trace, **kw)
    x = np.asarray(inputs["x"])
    B, _, W, H, L = x.shape
    outs = np.zeros((B, C, N), np.float32)
    for core in range(8):
        b, qb = core // 4, core % 4
        outs[b, :, qb * NQ:(qb + 1) * NQ] = res.results[core]["out"]
    return outs.reshape(B, C, W, H, L), res


def kernel(**inputs):
    out, _ = run_cores(inputs, trace=False)
    return out


# revision 51
# speedup vs baseline: 1.0166x; 1.0166x over previous
"""AttnBlock (GroupNorm + single-head full attention + residual) on 8 trn2 cores.

Sharding: core c in 0..7 handles batch b = c//4, query-block qb = c%4 (1024 of
4096 positions). Each core receives its batch's x with columns rotated so its
query block sits at columns 0:1023 (attention and groupnorm statistics are
invariant to a consistent permutation of key positions), computes the full
groupnorm + K/V for all 4096 positions, attention for its 1024 query positions,
and returns out[512, 1024]. The host gathers the 8 blocks.

On-device pipeline (all matmuls bf16 with fp32 PSUM accumulation):
  1. Stream x (fp32) through SBUF: per-channel sum / sum-of-squares for
     groupnorm stats (fp32), cast x to bf16 for the matmul path.
  2. Group stats via tiny one-hot matmuls across partitions; groupnorm is then
     folded into the QKV weights: h = a*x + bb  =>  W' = W * a (per input
     channel), bias' = W @ bb (+ original conv bias).
  3. q = W_q' x  [c, 1024];  k = W_k' x  [c, 4096];  vT = x^T W_v' [j, c]
     (v produced pre-transposed so the attention contraction over j needs no
     transposes anywhere).
  4. Per 512-wide query chunk: scoresT[j, i] = k^T q accumulated per 128-row
     j-tile in PSUM, exp on the scalar engine (softmax max-subtraction is
     skipped: logits are O(5) by construction), sum_j exp via ones-matmul,
     attn0[c, i] = vT^T p accumulated over all 32 j-tiles in PSUM.
  5. attn = attn0 / sum + v-path bias; proj = W_p attn + p_b + x (residual
     re-read from DRAM in fp32).
"""

import os
import sys

import numpy as np

for _p in ("/opt/trn_rl_repo", "/root/.axon_site/_ro/trn_rl_repo"):
    if os.path.isdir(_p) and _p not in sys.path:
        sys.path.insert(0, _p)

import ml_dtypes  # noqa: E402

import concourse.bacc as bacc  # noqa: E402
import concourse.bass as bass  # noqa: E402
import concourse.mybir as mybir  # noqa: E402
import concourse.tile as tile  # noqa: E402

F32 = mybir.dt.float32
BF16 = mybir.dt.bfloat16
FP8 = mybir.dt.float8e4
# fp8 attention-value path: p and vT quantized to e4m3, attnV + sumexp
# matmuls run in DoubleRow mode (2 contraction rows per PE cell -> half the
# matmul time). exp is biased by EXP_SHIFT so p fits e4m3 range; the shift
# cancels exactly in the softmax normalization.
FP8_ATTN = True
EXP_SHIFT = -2.0
AF = mybir.ActivationFunctionType
AX = mybir.AxisListType

P = 128
C = 512
CT = C // P            # 4 channel tiles
N = 4096               # key/value positions per batch
NQ = 1024              # query positions per core
ICH = 512              # query chunk (PSUM free dim)
NIC = NQ // ICH        # 2 query chunks
JT = N // P            # 32 key j-tiles
JC = N // 512          # 8 key j-chunks
NG = 32                # groupnorm groups
GS = C // NG           # 16 channels per group
EPS = 1e-6
NE = GS * N            # elements per group
SCALE = float(C) ** -0.5


def _emit(nc, tc, io):
    ctx = tc  # alias
    from contextlib import ExitStack

    es = ExitStack()
    wpool = es.enter_context(tc.tile_pool(name="w", bufs=4))
    cpool = es.enter_context(tc.tile_pool(name="consts", bufs=1))
    spool = es.enter_context(tc.tile_pool(name="stat", bufs=1))
    xbpool = es.enter_context(tc.tile_pool(name="xb", bufs=CT))
    kpool = es.enter_context(tc.tile_pool(name="k", bufs=CT))
    vpool = es.enter_context(tc.tile_pool(name="vt", bufs=JT))
    qpool = es.enter_context(tc.tile_pool(name="q", bufs=CT))
    sqpool = es.enter_context(tc.tile_pool(name="sq", bufs=2))
    ppool = es.enter_context(tc.tile_pool(name="p", bufs=4))
    apool = es.enter_context(tc.tile_pool(name="attn", bufs=8))
    anpool = es.enter_context(tc.tile_pool(name="anorm", bufs=2))
    rpool = es.enter_context(tc.tile_pool(name="rn", bufs=2))
    opool = es.enter_context(tc.tile_pool(name="osb", bufs=4))
    respool = es.enter_context(tc.tile_pool(name="res", bufs=1))
    psmm = es.enter_context(tc.tile_pool(name="psmm", bufs=4, space="PSUM"))
    pssc = es.enter_context(tc.tile_pool(name="pssc", bufs=3, space="PSUM"))
    pssum = es.enter_context(tc.tile_pool(name="pssum", bufs=1, space="PSUM"))

    xb16 = io["xb16"]
    xres = io["xres"]
    out = io["out"]

    # ---- phase B: x tiles first on the SP HWDGE queue (startup-critical);
    # everything else via gpsimd's software DGE so neither the SP queue nor
    # the ACT sequencer blocks on DMA ring credits.
    xb_sb = []
    s_tiles = []
    H = N // 2
    # x split between the SP HWDGE queue and gpsimd's SWDGE rings — both are
    # compute-free sequencers. The ACT queue must issue NO input DMAs: its
    # ring-credit waits would block all scalar-engine compute behind them.
    # 8 half-tiles over three rings (SP, ACT, SWDGE). The ACT queue gets only
    # 3 early DMAs — more would hit ring-credit waits that stall ACT compute.
    ring = [nc.sync, nc.scalar, nc.gpsimd,
            nc.sync, nc.scalar, nc.gpsimd,
            nc.sync, nc.scalar]
    for t in range(CT):
        xb = xbpool.tile([P, N], BF16, tag="xb", name=f"xb{t}")
        ring[2 * t].dma_start(xb[:, :H], xb16[t * P:(t + 1) * P, :H])
        ring[2 * t + 1].dma_start(xb[:, H:], xb16[t * P:(t + 1) * P, H:])
        xb_sb.append(xb)

    # ---- constants: small ones first (the stats matmuls need G early),
    # then the 4MB of weights, then the residual ---------------------------
    G_dma = cpool.tile([P, CT * NG], F32, tag="Gmd", name="Gmd")
    nc.sync.dma_start(G_dma, io["gmask"][:, :])
    G_sb = cpool.tile([P, CT * NG], F32, tag="Gm", name="Gm")
    # NOTE: the ACT copy of G is emitted AFTER the stats loop — engine streams
    # run in emission order, and an early-emitted copy waiting on the G DMA
    # (queued behind 4MB of x) would stall every ACT square behind it.
    GT_dma = cpool.tile([NG, C], F32, tag="GTmd", name="GTmd")
    nc.gpsimd.dma_start(GT_dma, io["gtmask"][:, :])
    GT_sb = cpool.tile([NG, C], F32, tag="GTm", name="GTm")
    nc.vector.tensor_copy(GT_sb, GT_dma)
    bias_all = cpool.tile([P, 24], F32, tag="bias_all", name="bias_all")
    nc.sync.dma_start(bias_all, io["bias6"][:, :])
    w_sb = {}
    for i, wn in enumerate(("wq", "wk", "wv", "wp")):
        wt = wpool.tile([P, CT, C], BF16, tag="w", name=f"{wn}_all")
        eng = nc.sync if i % 2 == 0 else nc.gpsimd
        eng.dma_start(wt, io[wn].rearrange("(t p) o -> p t o", p=P))
        w_sb[wn] = [wt[:, t, :] for t in range(CT)]
    # residual: DRAM-only dependency, needed only at the proj epilogue
    res_all = respool.tile([P, CT, NIC, ICH], F32, tag="res", name="res_all")
    nc.gpsimd.dma_start(
        res_all, xres.rearrange("(t p) (i n) -> p t i n", p=P, n=ICH))
    res_sb = [res_all[:, t, ic, :] for ic in range(NIC) for t in range(CT)]
    small = {}
    for idx, nm in enumerate(("qb2", "kb2", "vb2", "pb2", "gnw2", "gnb2")):
        small[nm] = bias_all[:, idx * CT:(idx + 1) * CT]
    ones_b = cpool.tile([P, 1], BF16, tag="ones_b", name="ones_b")
    nc.vector.memset(ones_b, 1.0)
    ones_p_t = cpool.tile([P, 2, 16], FP8, tag="ones_p", name="ones_p")
    nc.vector.memset(ones_p_t, 1.0)
    ones_p = ones_p_t[:, :, 0:1]  # pair stride 16 (DoubleRow needs step%16==0)
    nshift = cpool.tile([P, 1], F32, tag="nshift", name="nshift")
    nc.vector.memset(nshift, EXP_SHIFT)

    # ---- stats per half-tile (chases the DMA halves as they land) -------
    # s1 via DVE tensor_scalar+accum (bf16 2x mode, ~2x faster than reduce);
    # squares on ACT except the last tile's, which go to DVE STT so the two
    # engines finish together.
    for t in range(CT):
        xb = xb_sb[t]
        st = spool.tile([P, 2], F32, tag=f"s{t}", name=f"s{t}")
        hs = spool.tile([P, 4], F32, tag=f"hs{t}", name=f"hs{t}")
        for h in range(2):
            hsl = slice(h * H, (h + 1) * H)
            sq_scr = sqpool.tile([P, H], BF16, tag="sq", name=f"sq{t}_{h}")
            nc.scalar.activation(sq_scr, xb[:, hsl], AF.Square,
                                 accum_out=hs[:, 2 + h:3 + h])
            s1_scr = sqpool.tile([P, H], BF16, tag="s1s", name=f"s1s{t}_{h}")
            nc.vector.tensor_scalar(
                s1_scr, xb[:, hsl], 1.0, 0.0, mybir.AluOpType.mult,
                mybir.AluOpType.add, accum_out=hs[:, h:h + 1])
        nc.vector.tensor_add(st[:, 0:1], hs[:, 0:1], hs[:, 1:2])
        nc.vector.tensor_add(st[:, 1:2], hs[:, 2:3], hs[:, 3:4])
        s_tiles.append(st)
    nc.scalar.copy(G_sb, G_dma)

    # ---- phase C: group stats -------------------------------------------
    gs_ps = psmm.tile([NG, 2], F32, tag="mm", name="gsums")
    for t in range(CT):
        nc.tensor.matmul(gs_ps, lhsT=G_sb[:, t * NG:(t + 1) * NG],
                         rhs=s_tiles[t], start=(t == 0), stop=(t == CT - 1))
    vals = spool.tile([NG, 2], F32, tag="vals", name="vals")  # col0 rsig col1 mu
    ex2 = spool.tile([NG, 1], F32, tag="ex2", name="ex2")
    msq = spool.tile([NG, 1], F32, tag="msq", name="msq")
    sd = spool.tile([NG, 1], F32, tag="sd", name="sd")
    nc.vector.tensor_scalar_mul(vals[:, 1:2], gs_ps[:, 0:1], 1.0 / NE)
    nc.vector.tensor_scalar_mul(ex2, gs_ps[:, 1:2], 1.0 / NE)
    nc.vector.tensor_mul(msq, vals[:, 1:2], vals[:, 1:2])
    nc.vector.tensor_sub(msq, ex2, msq)
    nc.vector.tensor_scalar_add(msq, msq, EPS)
    nc.scalar.activation(sd, msq, AF.Sqrt)
    nc.vector.reciprocal_approx_fast(vals[:, 0:1], sd)

    # ---- phase D: per-channel a/bb, fold into weights -------------------
    a_t, bbb_t = [], []
    for t in range(CT):
        ch = psmm.tile([P, 2], F32, tag="mm", name=f"ch{t}")
        nc.tensor.matmul(ch, lhsT=GT_sb[:, t * P:(t + 1) * P], rhs=vals,
                         start=True, stop=True)
        at = spool.tile([P, 1], F32, tag=f"a{t}", name=f"a{t}")
        nc.vector.tensor_mul(at, ch[:, 0:1], small["gnw2"][:, t:t + 1])
        mt = spool.tile([P, 1], F32, tag=f"mt{t}", name=f"mt{t}")
        nc.vector.tensor_mul(mt, ch[:, 1:2], at)
        bbf = spool.tile([P, 1], F32, tag=f"bbf{t}", name=f"bbf{t}")
        nc.vector.tensor_sub(bbf, small["gnb2"][:, t:t + 1], mt)
        bbb = spool.tile([P, 1], BF16, tag=f"bbb{t}", name=f"bbb{t}")
        nc.vector.tensor_copy(bbb, bbf)
        a_t.append(at)
        bbb_t.append(bbb)

    # bias' = W @ bb (+ host conv bias); must read W before in-place scaling
    biases = {}
    for wn, hb in (("wq", "qb2"), ("wk", "kb2"), ("wv", "vb2")):
        bl = []
        for t in range(CT):
            bp = psmm.tile([P, 1], F32, tag="mm", name=f"B{wn}{t}")
            for ct in range(CT):
                nc.tensor.matmul(bp, lhsT=w_sb[wn][ct][:, t * P:(t + 1) * P],
                                 rhs=bbb_t[ct], start=(ct == 0),
                                 stop=(ct == CT - 1))
            bt = spool.tile([P, 1], F32, tag=f"bi{wn}{t}", name=f"bi{wn}{t}")
            nc.vector.tensor_add(bt, bp, small[hb][:, t:t + 1])
            bl.append(bt)
        biases[wn] = bl
    for wn in ("wq", "wk", "wv"):
        for ct in range(CT):
            nc.vector.tensor_scalar_mul(w_sb[wn][ct], w_sb[wn][ct], a_t[ct])

    # ---- phase E: q, then (k, vT) j-chunk-major -------------------------
    q_sb = [qpool.tile([P, NQ], BF16, tag="q", name=f"q{t}") for t in range(CT)]
    for t in range(CT):
        for ic in range(NIC):
            qp = psmm.tile([P, ICH], F32, tag="mm", name=f"qp{t}_{ic}")
            for ct in range(CT):
                nc.tensor.matmul(qp, lhsT=w_sb["wq"][ct][:, t * P:(t + 1) * P],
                                 rhs=xb_sb[ct][:, ic * ICH:(ic + 1) * ICH],
                                 start=(ct == 0), stop=(ct == CT - 1))
            nc.scalar.activation(q_sb[t][:, ic * ICH:(ic + 1) * ICH], qp,
                                 AF.Identity, bias=biases["wq"][t])
    k_sb = [kpool.tile([P, N], BF16, tag="k", name=f"k{t}") for t in range(CT)]
    vT_sb = []
    for jc in range(JC):
        sl = slice(jc * 512, (jc + 1) * 512)
        for t in range(CT):
            kp = psmm.tile([P, 512], F32, tag="mm", name=f"kp{t}_{jc}")
            for ct in range(CT):
                nc.tensor.matmul(kp, lhsT=w_sb["wk"][ct][:, t * P:(t + 1) * P],
                                 rhs=xb_sb[ct][:, sl],
                                 start=(ct == 0), stop=(ct == CT - 1))
            nc.scalar.activation(k_sb[t][:, sl], kp, AF.Identity,
                                 bias=biases["wk"][t])
        for jj in range(4):
            j = jc * 4 + jj
            vp = psmm.tile([P, C], F32, tag="mm", name=f"vp{j}")
            for ct in range(CT):
                nc.tensor.matmul(vp, lhsT=xb_sb[ct][:, j * P:(j + 1) * P],
                                 rhs=w_sb["wv"][ct],
                                 start=(ct == 0), stop=(ct == CT - 1))
            if FP8_ATTN:
                if j % 2 == 0:
                    vt = vpool.tile([P, 2, C], FP8, tag="vt", name=f"vt{j // 2}")
                    vT_sb.append(vt)
                nc.vector.tensor_copy(vT_sb[j // 2][:, j % 2, :], vp)
            else:
                vt = vpool.tile([P, C], BF16, tag="vt", name=f"vt{j}")
                nc.vector.tensor_copy(vt, vp)
                vT_sb.append(vt)

    # ---- phase F: attention per query chunk -----------------------------
    DR = mybir.MatmulPerfMode.DoubleRow
    attn_sb = [[None] * CT for _ in range(NIC)]
    for ic in range(NIC):
        isl = slice(ic * ICH, (ic + 1) * ICH)
        att_ps = [psmm.tile([P, ICH], F32, tag="mm", name=f"att{ic}_{c}")
                  for c in range(CT)]
        se_ps = pssum.tile([1, ICH], F32, tag="se", name=f"se{ic}")
        if FP8_ATTN:
            # Software-pipelined: emit pair g+1's scores before pair g's
            # DoubleRow matmuls. The DR ldweights carry the wait on exp(g)
            # (Bacc moves matmul waits to ldweights), and the PE is in-order,
            # so without the pipeline it idles ~exp-latency every pair.
            NPAIR = JT // 2
            pg_tiles = {}

            def emit_scores(g):
                pg = ppool.tile([P, 2, ICH], FP8, tag="p", name=f"p{ic}_{g}")
                for r in range(2):
                    j = 2 * g + r
                    sp = pssc.tile([P, ICH], F32, tag="sc", name=f"sp{ic}_{j}")
                    for ct in range(CT):
                        nc.tensor.matmul(
                            sp, lhsT=k_sb[ct][:, j * P:(j + 1) * P],
                            rhs=q_sb[ct][:, isl],
                            start=(ct == 0), stop=(ct == CT - 1))
                    nc.scalar.activation(pg[:, r, :], sp, AF.Exp,
                                         bias=nshift, scale=SCALE)
                pg_tiles[g] = pg

            emit_scores(0)
            for g in range(NPAIR):
                if g + 1 < NPAIR:
                    emit_scores(g + 1)
                pg = pg_tiles.pop(g)
                nc.tensor.matmul(se_ps, lhsT=ones_p, rhs=pg, perf_mode=DR,
                                 start=(g == 0), stop=(g == NPAIR - 1))
                for c in range(CT):
                    nc.tensor.matmul(
                        att_ps[c], lhsT=vT_sb[g][:, :, c * P:(c + 1) * P],
                        rhs=pg, perf_mode=DR,
                        start=(g == 0), stop=(g == NPAIR - 1))
        else:
            for j in range(JT):
                sp = pssc.tile([P, ICH], F32, tag="sc", name=f"sp{ic}_{j}")
                for ct in range(CT):
                    nc.tensor.matmul(sp, lhsT=k_sb[ct][:, j * P:(j + 1) * P],
                                     rhs=q_sb[ct][:, isl],
                                     start=(ct == 0), stop=(ct == CT - 1))
                pj = ppool.tile([P, ICH], BF16, tag="p", name=f"p{ic}_{j}")
                nc.scalar.activation(pj, sp, AF.Exp, scale=SCALE)
                nc.tensor.matmul(se_ps, lhsT=ones_b, rhs=pj,
                                 start=(j == 0), stop=(j == JT - 1))
                for c in range(CT):
                    nc.tensor.matmul(att_ps[c],
                                     lhsT=vT_sb[j][:, c * P:(c + 1) * P],
                                     rhs=pj, start=(j == 0), stop=(j == JT - 1))
        r_sb = rpool.tile([1, ICH], F32, tag="r", name=f"r{ic}")
        nc.vector.reciprocal_approx_fast(r_sb, se_ps)
        # [1,512] -> [128,512] partition broadcast on gpsimd (keeps PE free)
        rbc = rpool.tile([P, ICH], F32, tag="rbc", name=f"rbc{ic}")
        nc.gpsimd.partition_broadcast(rbc, r_sb)
        for c in range(CT):
            an = anpool.tile([P, ICH], F32, tag="an", name=f"an{ic}_{c}")
            nc.vector.tensor_mul(an, att_ps[c], rbc)
            at = apool.tile([P, ICH], BF16, tag="attn", name=f"at{ic}_{c}")
            nc.scalar.activation(at, an, AF.Identity, bias=biases["wv"][c])
            attn_sb[ic][c] = at

    # ---- phase G: proj + residual + store -------------------------------
    for ic in range(NIC):
        isl = slice(ic * ICH, (ic + 1) * ICH)
        for t in range(CT):
            op_ps = pssc.tile([P, ICH], F32, tag="sc", name=f"op{ic}_{t}")
            for ct in range(CT):
                nc.tensor.matmul(op_ps, lhsT=w_sb["wp"][ct][:, t * P:(t + 1) * P],
                                 rhs=attn_sb[ic][ct],
                                 start=(ct == 0), stop=(ct == CT - 1))
            osb = opool.tile([P, ICH], F32, tag="o", name=f"o{ic}_{t}")
            nc.vector.scalar_tensor_tensor(
                osb, in0=op_ps, scalar=small["pb2"][:, t:t + 1],
                in1=res_sb[ic * CT + t],
                op0=mybir.AluOpType.add, op1=mybir.AluOpType.add)
            eng = nc.sync if t % 2 == 0 else nc.scalar
            eng.dma_start(out[t * P:(t + 1) * P, isl], osb)
    es.close()


def build_nc():
    nc = bacc.Bacc("TRN2", target_bir_lowering=False, debug=False)
    io = {}
    io["xb16"] = nc.dram_tensor("xb16", [C, N], BF16, kind="ExternalInput").ap()
    io["xres"] = nc.dram_tensor("xres", [C, NQ], F32, kind="ExternalInput").ap()
    for wn in ("wq", "wk", "wv", "wp"):
        io[wn] = nc.dram_tensor(wn, [C, C], BF16, kind="ExternalInput").ap()
    io["bias6"] = nc.dram_tensor("bias6", [P, 24], F32,
                                 kind="ExternalInput").ap()
    io["gmask"] = nc.dram_tensor("gmask", [P, CT * NG], F32,
                                 kind="ExternalInput").ap()
    io["gtmask"] = nc.dram_tensor("gtmask", [NG, C], F32,
                                  kind="ExternalInput").ap()
    io["out"] = nc.dram_tensor("out", [C, NQ], F32, kind="ExternalOutput").ap()
    with tile.TileContext(nc) as tc:
        _emit(nc, tc, io)
    nc.compile()
    return nc


def make_in_maps(inputs):
    bf = ml_dtypes.bfloat16
    x = np.asarray(inputs["x"], np.float32)
    B = x.shape[0]
    bias6 = np.concatenate(
        [np.asarray(inputs[nm], np.float32).reshape(CT, P).T
         for nm in ("q_b", "k_b", "v_b", "p_b", "gn_w", "gn_b")], axis=1)
    shared = {
        "wq": np.ascontiguousarray(np.asarray(inputs["q_w"], np.float32).T).astype(bf),
        "wk": np.ascontiguousarray(np.asarray(inputs["k_w"], np.float32).T).astype(bf),
        "wv": np.ascontiguousarray(np.asarray(inputs["v_w"], np.float32).T).astype(bf),
        "wp": np.ascontiguousarray(np.asarray(inputs["p_w"], np.float32).T).astype(bf),
        "bias6": np.ascontiguousarray(bias6),
    }
    # one-hot group masks: channel k of c-tile t belongs to group (t*128+k)//16
    gm = np.zeros((P, CT, NG), np.float32)
    for t in range(CT):
        for k in range(P):
            gm[k, t, (t * P + k) // GS] = 1.0
    shared["gmask"] = np.ascontiguousarray(gm.reshape(P, CT * NG))
    gt = np.zeros((NG, C), np.float32)
    for ch in range(C):
        gt[ch // GS, ch] = 1.0
    shared["gtmask"] = gt
    in_maps = []
    for core in range(8):
        b, qb = core // 4, core % 4
        xb = x[b].reshape(C, N)
        xp = np.ascontiguousarray(np.roll(xb, -qb * NQ, axis=1))
        in_maps.append({**shared,
                        "xb16": xp.astype(bf),
                        "xres": np.ascontiguousarray(xp[:, :NQ])})
    return in_maps


_NC_CACHE = {}


def run_cores(inputs, trace=False, **kw):
    from concourse.bass_utils import run_bass_kernel_spmd
    if "nc" not in _NC_CACHE:
        _NC_CACHE["nc"] = build_nc()
    nc = _NC_CACHE["nc"]
    in_maps = make_in_maps(inputs)
    res = run_bass_kernel_spmd(nc, in_maps, core_ids=list(range(8)),
                               trace=trace, **kw)
    x = np.asarray(inputs["x"])
    B, _, W, H, L = x.shape
    outs = np.zeros((B, C, N), np.float32)
    for core in range(8):
        b, qb = core // 4, core % 4
        outs[b, :, qb * NQ:(qb + 1) * NQ] = res.results[core]["out"]
    return outs.reshape(B, C, W, H, L), res


def kernel(**inputs):
    out, _ = run_cores(inputs, trace=False)
    return out
